# revision 17
# baseline (speedup 1.0000x reference)
"""Trainium2 Bass kernel for nn_Attention_7078106104284.

Self-attention block (SAGAN-style) over x[8, 256, 64, 64]:
  q = wq@x+bq [32,n], k = wk@x+bk [32,n], v = wv@x+bv [256,n], n = 4096
  attn = softmax(q^T k, axis=m);  y = x + gamma * (v @ attn^T)

Sharding: data-parallel over batch - one batch element per NeuronCore (8 cores).

All compute in plain bf16 (correctness gate is rel_err < 2e-2; bf16 inputs
with fp32 PSUM accumulation lands ~3e-3). Design notes:

  - q/k projections: K=256 bf16 accumulating matmuls per 512-wide n-tile;
    bias fused into the ACT eviction; results replicated x4 along SBUF
    partitions (strips at 0/32/64/96) via SBUF->SBUF DMA so logit matmuls
    can be row-tiled.
  - Logits Lt[m,n] = sum_o k[o,m] q[o,n]: K=32 matmuls packed 4-per-quad
    with tile_position=(32i,0) - four m-chunks compute concurrently in
    disjoint 32-row strips of the PE array, each into its own PSUM bank.
  - exp fused with PSUM->SBUF evacuation on ACT as ONE FD=2048 call per
    quad, bf16 out. Softmax max-subtraction skipped: |logit| < 50 << 88.
  - vT[m, c'] bf16 with a ones column at c' = 256 (tiny K=1 matmul) so the
    softmax denominator Z rides the AV product for free.
  - AV transposed: uT[n, c'] = sum_m e[m, n] vT[m, c'] per 128-wide n-sub
    (stationary = e chunk, moving = vT, N=257).
  - Normalize on DVE: uTn = uT * (gamma/Z[n]) per-partition, bf16; PE
    transposes uTn back to [c, n]; epilogue y = trans + xp where
    xp = x + gamma*bv precomputed once (v's bias folds into the residual
    because sum_m attn = 1).
"""

import sys

sys.path.insert(0, "/opt/trn_rl_repo")

import numpy as np
from contextlib import ExitStack

import concourse.bass as bass
import concourse.bacc as bacc
import concourse.tile as tile
import concourse.mybir as mybir
from concourse.masks import make_identity
from concourse.bass_utils import run_bass_kernel_spmd

dt = mybir.dt
AF = mybir.ActivationFunctionType

B = 8
C = 256
C8 = 32
N = 4096          # h*w spatial positions
NG = 512          # n-group width (one PSUM bank of fp32)
G = N // NG       # 8 n-groups
MC = N // 128     # 32 m-chunks
EW = 4            # m-chunks per quad (PSUM banks per plt tile)
RND = MC // EW    # quads (rounds) per group
CP = C + 1        # AV output channels incl. the Z ones-column
A_SCH = 128 * np.log2(np.e)   # Schraudolph logit pre-scale (folded into k)
# quads whose exp runs on DVE via the int16 trick (ACT/DVE load balance)
DVE_QUADS = frozenset({(g, j) for g in range(G) for j in (2, 5)}
                      | {(g, 7) for g in (1, 3, 5)})


def build_program(reps=1, ablate=()):
    nc = bacc.Bacc("TRN2", target_bir_lowering=False)
    f32 = dt.float32
    bf16 = dt.bfloat16
    x_d = nc.declare_dram_parameter("x", [C, N], f32, isOutput=False)
    xhi_d = nc.declare_dram_parameter("x_hi", [C, N], bf16, isOutput=False)
    wq_d = nc.declare_dram_parameter("wqT", [C, C8], bf16, isOutput=False)
    wk_d = nc.declare_dram_parameter("wkT", [C, C8], bf16, isOutput=False)
    wv_d = nc.declare_dram_parameter("wvT", [C, C], bf16, isOutput=False)
    bq_d = nc.declare_dram_parameter("bq", [C8, 1], f32, isOutput=False)
    bk_d = nc.declare_dram_parameter("bk", [C8, 1], f32, isOutput=False)
    bv_d = nc.declare_dram_parameter("gbv", [128, 2], f32, isOutput=False)
    y_d = nc.declare_dram_parameter("y", [C, N], f32, isOutput=True)

    with tile.TileContext(nc) as tc, ExitStack() as ctx:
        sing = ctx.enter_context(tc.tile_pool(name="sing", bufs=1))
        epool = ctx.enter_context(tc.tile_pool(name="epool", bufs=RND + 2))
        upool = ctx.enter_context(tc.tile_pool(name="upool", bufs=6))
        ypool = ctx.enter_context(tc.tile_pool(name="ypool", bufs=3))
        scal = ctx.enter_context(tc.tile_pool(name="scal", bufs=4))

        lt_ps = ctx.enter_context(tc.tile_pool(name="lt_ps", bufs=1, space="PSUM"))
        u_ps = ctx.enter_context(tc.tile_pool(name="u_ps", bufs=1, space="PSUM"))

        for _rep in range(reps):
            # ---- static inputs ----
            x_sb = sing.tile([128, 2, N], f32)           # residual term
            nc.sync.dma_start(out=x_sb, in_=x_d[:].rearrange("(cc p) m -> p cc m", p=128))
            xhi_sb = sing.tile([128, 2, N], bf16)
            nc.sync.dma_start(out=xhi_sb, in_=xhi_d[:].rearrange("(cc p) m -> p cc m", p=128))
            wq_sb = sing.tile([128, 2, C8], bf16)        # slots cc: 0, 1
            nc.sync.dma_start(out=wq_sb, in_=wq_d[:].rearrange("(cc p) o -> p cc o", p=128))
            wk_sb = sing.tile([128, 2, C8], bf16)
            nc.sync.dma_start(out=wk_sb, in_=wk_d[:].rearrange("(cc p) o -> p cc o", p=128))
            wv_sb = sing.tile([128, 2, C], bf16)
            nc.sync.dma_start(out=wv_sb, in_=wv_d[:].rearrange("(cc p) c -> p cc c", p=128))
            bq_sb = sing.tile([C8, 1], f32)
            nc.sync.dma_start(out=bq_sb, in_=bq_d[:])
            bk_sb = sing.tile([C8, 1], f32)
            nc.sync.dma_start(out=bk_sb, in_=bk_d[:])
            bv_sb = sing.tile([128, 2], f32)   # gamma*bv, pre-scaled on host
            nc.sync.dma_start(out=bv_sb, in_=bv_d[:])

            ones_f32 = sing.tile([128, 1], f32)
            nc.vector.memset(ones_f32, 1.0)
            # Schraudolph bias: e^l ~ bf16_bits(int16(l*128*log2e + 16256-C)).
            # k (and bk) are pre-scaled by a=128*log2e on the host, so PSUM
            # logits arrive as l*a; ACT-exp quads undo it via scale=1/a.
            bsch = sing.tile([128, 1], f32)
            nc.vector.memset(bsch, 16256.0 - 5.59)
            one_b = sing.tile([1, 1], bf16)              # K=1 ones-column writer
            nc.scalar.activation(one_b, ones_f32[0:1, :], AF.Copy)
            one_row_b = sing.tile([1, 128], bf16)
            nc.scalar.activation(
                one_row_b, bass.AP(tensor=ones_f32.tensor, offset=ones_f32.offset,
                                   ap=[[1, 1], [0, 128]]), AF.Copy)
            ident = sing.tile([128, 128], bf16)          # transpose identity
            make_identity(nc, ident)

            # xp = x + gamma*bv (per-partition adder folded into residual;
            # v's bias folds in because sum_m attn = 1). On GPSIMD: the one
            # SBUF-only elementwise op, keeps DVE free for exp quads.
            for cb in range(2):
                nc.gpsimd.tensor_scalar_add(x_sb[:, cb, :], x_sb[:, cb, :],
                                            bv_sb[:, cb:cb + 1])

            # ---- q/k projections, replicated x4 along partition strips ----
            q4 = sing.tile([128, N], bf16)   # strips at partitions 0/32/64/96
            k4 = sing.tile([128, N], bf16)
            for s in range(G):
                sl = slice(s * NG, (s + 1) * NG)
                pq = u_ps.tile([C8, NG], f32, tag="u0", name="pq")
                pk = u_ps.tile([C8, NG], f32, tag="u1", name="pk")
                for cc in range(2):
                    nc.tensor.matmul(pq, wq_sb[:, cc, :], xhi_sb[:, cc, sl],
                                     start=(cc == 0), stop=(cc == 1))
                for cc in range(2):
                    nc.tensor.matmul(pk, wk_sb[:, cc, :], xhi_sb[:, cc, sl],
                                     start=(cc == 0), stop=(cc == 1))
                nc.vector.tensor_scalar_add(q4[0:C8, sl], pq, bq_sb)
                nc.vector.tensor_scalar_add(k4[0:C8, sl], pk, bk_sb)
            # replicate to strips 1..3 (SBUF->SBUF DMA shifts partitions)
            for st in range(1, 4):
                nc.sync.dma_start(out=q4[st * C8:(st + 1) * C8, :], in_=q4[0:C8, :])
                nc.sync.dma_start(out=k4[st * C8:(st + 1) * C8, :], in_=k4[0:C8, :])

            # ---- vT[m, c'] in bf16 with ones column at c' = 256 ----
            vt_sb = sing.tile([128, MC, CP], bf16)
            for mc in range(MC):
                msl = slice(mc * 128, (mc + 1) * 128)
                pv = u_ps.tile([128, CP], f32, tag="u2", name="pv")
                for cc in range(2):
                    nc.tensor.matmul(pv[:, 0:C], xhi_sb[:, cc, msl], wv_sb[:, cc, :],
                                     start=(cc == 0), stop=(cc == 1))
                nc.tensor.matmul(pv[:, C:CP], one_row_b, one_b,
                                 start=True, stop=True)
                nc.vector.tensor_copy(vt_sb[:, mc, :], pv)

            # ---- attention, software-pipelined over n-groups ----
            e_tiles = {}
            u_tiles = {}

            def issue_lt_exp(g, j):
                # quad j: logits for m-chunks 4j..4j+3, row-tiled 4x
                # concurrent (tile_position=(32i,0)), each into its own
                # PSUM bank; one fused FD=2048 exp over all four.
                sl = slice(g * NG, (g + 1) * NG)
                plt = lt_ps.tile([128, EW, NG], f32, tag="plt", name="plt")
                for rg in range(EW if "lt" not in ablate else 1):
                    mc = EW * j + rg
                    msl = slice(mc * 128, (mc + 1) * 128)
                    nc.tensor.matmul(plt[:, rg, :],
                                     k4[rg * C8:(rg + 1) * C8, msl],
                                     q4[rg * C8:(rg + 1) * C8, sl],
                                     start=True, stop=True,
                                     tile_position=(rg * C8, 0))
                e_t = epool.tile([128, EW, NG], bf16, tag="e", name="e_t")
                if "noexp" in ablate:
                    nc.gpsimd.memset(e_t, 0.5)
                elif (g, j) in DVE_QUADS:
                    nc.vector.tensor_scalar_add(
                        e_t[:, :, :].bitcast(dt.int16), plt, bsch)
                else:
                    fn = AF.Exp if "exp" not in ablate else AF.Copy
                    nc.scalar.activation(e_t, plt, fn, scale=float(1.0 / A_SCH))
                e_tiles[(g, j)] = e_t

            def issue_av(g, j):
                uts = u_tiles[g]
                e_t = e_tiles.pop((g, j))
                if "av" in ablate:
                    if j == 0:
                        for sub in range(4):
                            nc.tensor.matmul(uts[sub],
                                             e_t[:, 0, sub * 128:(sub + 1) * 128],
                                             vt_sb[:, 0, :], start=True, stop=True)
                    return
                for rg in range(EW):
                    mc = EW * j + rg
                    first = (j == 0 and rg == 0)
                    last = (j == RND - 1 and rg == EW - 1)
                    for sub in range(4):
                        nc.tensor.matmul(uts[sub],
                                         e_t[:, rg, sub * 128:(sub + 1) * 128],
                                         vt_sb[:, mc, :],
                                         start=first, stop=last)

            def issue_epilogue(g):
                uts = u_tiles.pop(g)
                # normalize per n-sub-block to bf16, transpose back to
                # [c, n] on PE (bf16 fast path); the two transpose PSUM
                # tiles reuse freed u0/u1 slots. y = trans + xp.
                tph = [u_ps.tile([128, NG], dt.bfloat16, tag=f"u{cb}", name="tph")
                       for cb in range(2)]
                for sub in range(4):
                    ut = uts[sub]
                    rinv = scal.tile([128, 1], f32, tag="rinv", name="rinv")
                    nc.vector.reciprocal(rinv, ut[:, C:CP])
                    un = upool.tile([128, C], dt.bfloat16, tag="un", name="un")
                    nc.vector.tensor_scalar_mul(un, ut[:, 0:C], rinv)
                    for cb in range(2):
                        nc.tensor.transpose(
                            tph[cb][:, sub * 128:(sub + 1) * 128],
                            un[:, cb * 128:(cb + 1) * 128], ident)
                sl = slice(g * NG, (g + 1) * NG)
                for cb in range(2):
                    y_t = ypool.tile([128, NG], f32, tag="y", name="y")
                    nc.vector.tensor_add(y_t, tph[cb], x_sb[:, cb, sl])
                    nc.sync.dma_start(
                        out=y_d[:].rearrange("(cc p) m -> p cc m", p=128)[:, cb, sl],
                        in_=y_t,
                    )

            for g in range(G + 1):
                if g < G:
                    u_tiles[g] = [u_ps.tile([128, CP], f32, tag=f"u{s}", name=f"u{s}")
                                  for s in range(4)]
                for j in range(RND):
                    if g < G:
                        issue_lt_exp(g, j)
                    if g >= 1:
                        issue_av(g - 1, j)
                if g >= 1:
                    issue_epilogue(g - 1)

    nc.compile()
    return nc


def prep_in_maps(inputs):
    """Full inputs dict -> per-core in_maps for run_bass_kernel_spmd."""
    import ml_dtypes
    bf = ml_dtypes.bfloat16
    x = np.asarray(inputs["x"], dtype=np.float32)
    xr = np.ascontiguousarray(x.reshape(B, C, N))
    x_hi = xr.astype(bf)
    shared = {
        "wqT": np.ascontiguousarray(
            np.asarray(inputs["wq"], np.float32).T.astype(bf)),
        "wkT": np.ascontiguousarray(
            (np.float32(A_SCH)
             * np.asarray(inputs["wk"], np.float32)).T.astype(bf)),
        "wvT": np.ascontiguousarray(
            (np.asarray(inputs["gamma"], np.float32).reshape(())
             * np.asarray(inputs["wv"], np.float32)).T.astype(bf)),
        "bq": np.ascontiguousarray(
            np.asarray(inputs["bq"], np.float32).reshape(C8, 1)),
        "bk": np.ascontiguousarray(
            (np.float32(A_SCH)
             * np.asarray(inputs["bk"], np.float32)).reshape(C8, 1)),
        "gbv": np.ascontiguousarray(
            (np.asarray(inputs["gamma"], np.float32).reshape(())
             * np.asarray(inputs["bv"], np.float32)).reshape(2, 128).T),
    }
    return [dict(shared, x=xr[i], x_hi=np.ascontiguousarray(x_hi[i]))
            for i in range(B)]


_nc_cache = None


def kernel(**inputs) -> np.ndarray:
    global _nc_cache
    if _nc_cache is None:
        _nc_cache = build_program()
    nc = _nc_cache
    in_maps = prep_in_maps(inputs)
    res = run_bass_kernel_spmd(nc, in_maps, core_ids=list(range(B)))
    y = np.stack([res.results[i]["y"] for i in range(B)], axis=0)
    return y.reshape(B, C, 64, 64).astype(np.float32)


if __name__ == "__main__":
    rng = np.random.default_rng(0)
    ins = {
        "x": rng.standard_normal((B, C, 64, 64), dtype=np.float32),
        "wq": rng.standard_normal((C8, C), dtype=np.float32) / 16,
        "bq": rng.standard_normal((C8,), dtype=np.float32) * 0.01,
        "wk": rng.standard_normal((C8, C), dtype=np.float32) / 16,
        "bk": rng.standard_normal((C8,), dtype=np.float32) * 0.01,
        "wv": rng.standard_normal((C, C), dtype=np.float32) / 16,
        "bv": rng.standard_normal((C,), dtype=np.float32) * 0.01,
        "gamma": rng.standard_normal((1,), dtype=np.float32) * 0.1,
    }
    out = kernel(**ins)
    print("kernel output", out.shape, out.dtype)


# revision 41
# speedup vs baseline: 1.2857x; 1.2857x over previous
"""Trainium2 Bass kernel for nn_Attention_7078106104284.

Self-attention block (SAGAN-style) over x[8, 256, 64, 64]:
  q = wq@x+bq [32,n], k = wk@x+bk [32,n], v = wv@x+bv [256,n], n = 4096
  attn = softmax(q^T k, axis=m);  y = x + gamma * (v @ attn^T)

Sharding: data-parallel over batch - one batch element per NeuronCore (8 cores).

All compute in plain bf16 (correctness gate is rel_err < 2e-2; bf16 inputs
with fp32 PSUM accumulation lands ~3e-3). Design notes:

  - q/k projections: K=256 bf16 accumulating matmuls per 512-wide n-tile,
    q and k col-tiled into disjoint 32-col PE strips (concurrent); bias
    fused into the DVE eviction; results replicated x4 along SBUF
    partitions (strips at 0/32/64/96) via SBUF->SBUF DMA so logit matmuls
    can be row-tiled. gamma is folded into wv/bv on the host; k is
    pre-scaled by a=128*log2e (Schraudolph-ready logits; ACT exp uses
    scale=1/a).
  - Logits Lt[m,n] = sum_o k[o,m] q[o,n]: K=32 matmuls packed 4-per-quad
    with tile_position=(32i,0) - four m-chunks compute concurrently in
    disjoint 32-row strips of the PE array, each into its own PSUM bank.
  - exp fused with PSUM->SBUF evacuation on ACT as ONE FD=2048 call per
    quad, bf16 out. Softmax max-subtraction skipped: |logit| < 50 << 88.
    (Optional dve_quads: exp via int16-Schraudolph on DVE - measured
    neutral under sustained load, default off.)
  - vT[m, c'] bf16 with a ones column at c' = 256 (DVE memset) so the
    softmax denominator Z rides the AV product for free.
  - AV transposed: uT[n, c'] = sum_m e[m, n] vT[m, c'] per 128-wide n-sub
    (stationary = e chunk w/ fast-weight-load, moving = vT, N=257).
  - Normalize on DVE: uTn = uT * (1/Z[n]) per-partition, bf16; PE
    transposes uTn back to [c, n] (transpose-mode); epilogue fuses
    y = tph + gamma*bv + x_hi in one scalar_tensor_tensor op (v's bias
    folds into the residual because sum_m attn = 1).

Measured pitfalls (see memory): GPSIMD is ~25x slower than DVE on large
tiles; DMA-xbar transposes serialize the epilogue (lose ~70us vs PE);
timing must be compared via same-session interleaved A/B (thermal drift).
"""

import sys

sys.path.insert(0, "/opt/trn_rl_repo")

import numpy as np
from contextlib import ExitStack

import concourse.bass as bass
import concourse.bacc as bacc
import concourse.tile as tile
import concourse.mybir as mybir
from concourse.masks import make_identity
from concourse.bass_utils import run_bass_kernel_spmd

dt = mybir.dt
AF = mybir.ActivationFunctionType

B = 8
C = 256
C8 = 32
N = 4096          # h*w spatial positions
NG = 512          # n-group width (one PSUM bank of fp32)
G = N // NG       # 8 n-groups
MC = N // 128     # 32 m-chunks
EW = 4            # m-chunks per quad (PSUM banks per plt tile)
RND = MC // EW    # quads (rounds) per group
CP = C + 1        # AV output channels incl. the Z ones-column
A_SCH = 128 * np.log2(np.e)   # Schraudolph logit pre-scale (folded into k)
# quads whose exp runs on DVE via the int16 Schraudolph trick. Measured:
# no benefit under sustained load (PE-bound; ACT has slack) - default off.
DVE_QUADS = frozenset()


def build_program(reps=1, ablate=(), dve_quads=DVE_QUADS, pe_transpose=True,
                  xp_engine="vector", ebufs=16):
    nc = bacc.Bacc("TRN2", target_bir_lowering=False)
    f32 = dt.float32
    bf16 = dt.bfloat16
    xhi_d = nc.declare_dram_parameter("x_hi", [C, N], bf16, isOutput=False)
    wq_d = nc.declare_dram_parameter("wqT", [C, C8], bf16, isOutput=False)
    wk_d = nc.declare_dram_parameter("wkT", [C, C8], bf16, isOutput=False)
    wv_d = nc.declare_dram_parameter("wvT", [C, C], bf16, isOutput=False)
    bq_d = nc.declare_dram_parameter("bq", [C8, 1], f32, isOutput=False)
    bk_d = nc.declare_dram_parameter("bk", [C8, 1], f32, isOutput=False)
    bv_d = nc.declare_dram_parameter("gbv", [128, 2], f32, isOutput=False)
    y_d = nc.declare_dram_parameter("y", [C, N], f32, isOutput=True)

    with tile.TileContext(nc) as tc, ExitStack() as ctx:
        sing = ctx.enter_context(tc.tile_pool(name="sing", bufs=1))
        epool = ctx.enter_context(tc.tile_pool(name="epool", bufs=ebufs))
        upool = ctx.enter_context(tc.tile_pool(name="upool", bufs=6))
        ypool = ctx.enter_context(tc.tile_pool(name="ypool", bufs=3))
        scal = ctx.enter_context(tc.tile_pool(name="scal", bufs=4))

        lt_ps = ctx.enter_context(tc.tile_pool(name="lt_ps", bufs=1, space="PSUM"))
        u_ps = ctx.enter_context(tc.tile_pool(name="u_ps", bufs=1, space="PSUM"))

        for _rep in range(reps):
            # ---- static inputs ----
            xhi_sb = sing.tile([128, 2, N], bf16)
            nc.sync.dma_start(out=xhi_sb, in_=xhi_d[:].rearrange("(cc p) m -> p cc m", p=128))
            wq_sb = sing.tile([128, 2, C8], bf16)        # slots cc: 0, 1
            nc.sync.dma_start(out=wq_sb, in_=wq_d[:].rearrange("(cc p) o -> p cc o", p=128))
            wk_sb = sing.tile([128, 2, C8], bf16)
            nc.sync.dma_start(out=wk_sb, in_=wk_d[:].rearrange("(cc p) o -> p cc o", p=128))
            wv_sb = sing.tile([128, 2, C], bf16)
            nc.sync.dma_start(out=wv_sb, in_=wv_d[:].rearrange("(cc p) c -> p cc c", p=128))
            bq_sb = sing.tile([C8, 1], f32)
            nc.sync.dma_start(out=bq_sb, in_=bq_d[:])
            bk_sb = sing.tile([C8, 1], f32)
            nc.sync.dma_start(out=bk_sb, in_=bk_d[:])
            bv_sb = sing.tile([128, 2], f32)   # gamma*bv, pre-scaled on host
            nc.sync.dma_start(out=bv_sb, in_=bv_d[:])

            # Schraudolph bias: e^l ~ bf16_bits(int16(l*128*log2e + 16256-C)).
            # k (and bk) are pre-scaled by a=128*log2e on the host, so PSUM
            # logits arrive as l*a; ACT-exp quads undo it via scale=1/a.
            bsch = sing.tile([128, 1], f32)
            nc.vector.memset(bsch, 16256.0 - 5.59)
            if pe_transpose:
                ident = sing.tile([128, 128], bf16)
                make_identity(nc, ident)

            # Residual y = attn_out + x_hi + gamma*bv is fused into the
            # epilogue via scalar_tensor_tensor (v's bias folds into the
            # residual because sum_m attn = 1) - no staged xp needed.

            # ---- q/k projections, replicated x4 along partition strips ----
            q4 = sing.tile([128, N], bf16)   # strips at partitions 0/32/64/96
            k4 = sing.tile([128, N], bf16)
            for s in range(G):
                sl = slice(s * NG, (s + 1) * NG)
                # q and k col-tiled into disjoint 32-col strips of the PE
                # array - the two M=32 matmuls run concurrently.
                pqk = u_ps.tile([2 * C8, NG], f32, tag="u0", name="pqk")
                for cc in range(2):
                    nc.tensor.matmul(pqk[0:C8, :], wq_sb[:, cc, :],
                                     xhi_sb[:, cc, sl],
                                     start=(cc == 0), stop=(cc == 1),
                                     tile_position=(0, 0))
                    nc.tensor.matmul(pqk[C8:2 * C8, :], wk_sb[:, cc, :],
                                     xhi_sb[:, cc, sl],
                                     start=(cc == 0), stop=(cc == 1),
                                     tile_position=(0, 32))
                nc.vector.tensor_scalar_add(q4[0:C8, sl], pqk[0:C8, :], bq_sb)
                nc.vector.tensor_scalar_add(k4[0:C8, sl], pqk[C8:2 * C8, :],
                                            bk_sb)
            # replicate to strips 1..3 (SBUF->SBUF DMA shifts partitions)
            for st in range(1, 4):
                nc.sync.dma_start(out=q4[st * C8:(st + 1) * C8, :], in_=q4[0:C8, :])
                nc.sync.dma_start(out=k4[st * C8:(st + 1) * C8, :], in_=k4[0:C8, :])

            # ---- vT[m, c'] in bf16 with ones column at c' = 256 ----
            vt_sb = sing.tile([128, MC, CP], bf16)
            nc.vector.memset(vt_sb[:, :, C:CP], 1.0)   # Z ones-column
            for mc in range(MC):
                msl = slice(mc * 128, (mc + 1) * 128)
                pv = u_ps.tile([128, C], f32, tag="u2", name="pv")
                for cc in range(2):
                    nc.tensor.matmul(pv, xhi_sb[:, cc, msl], wv_sb[:, cc, :],
                                     start=(cc == 0), stop=(cc == 1))
                nc.vector.tensor_copy(vt_sb[:, mc, 0:C], pv)

            # ---- attention, software-pipelined over n-groups ----
            e_tiles = {}
            u_tiles = {}

            def issue_lt_exp(g, j):
                # quad j: logits for m-chunks 4j..4j+3, row-tiled 4x
                # concurrent (tile_position=(32i,0)), each into its own
                # PSUM bank; one fused FD=2048 exp over all four.
                sl = slice(g * NG, (g + 1) * NG)
                plt = lt_ps.tile([128, EW, NG], f32, tag="plt", name="plt")
                for rg in range(EW if "lt" not in ablate else 1):
                    mc = EW * j + rg
                    msl = slice(mc * 128, (mc + 1) * 128)
                    nc.tensor.matmul(plt[:, rg, :],
                                     k4[rg * C8:(rg + 1) * C8, msl],
                                     q4[rg * C8:(rg + 1) * C8, sl],
                                     start=True, stop=True,
                                     tile_position=(rg * C8, 0))
                e_t = epool.tile([128, EW, NG], bf16, tag="e", name="e_t")
                if "noexp" in ablate:
                    nc.gpsimd.memset(e_t, 0.5)
                elif (g, j) in dve_quads:
                    nc.vector.tensor_scalar_add(
                        e_t[:, :, :].bitcast(dt.int16), plt, bsch)
                else:
                    fn = AF.Exp if "exp" not in ablate else AF.Copy
                    nc.scalar.activation(e_t, plt, fn, scale=float(1.0 / A_SCH))
                e_tiles[(g, j)] = e_t

            def issue_av(g, j):
                uts = u_tiles[g]
                e_t = e_tiles.pop((g, j))
                if "av" in ablate:
                    if j == 0:
                        for sub in range(4):
                            nc.tensor.matmul(uts[sub],
                                             e_t[:, 0, sub * 128:(sub + 1) * 128],
                                             vt_sb[:, 0, :], start=True, stop=True)
                    return
                for rg in range(EW):
                    mc = EW * j + rg
                    first = (j == 0 and rg == 0)
                    last = (j == RND - 1 and rg == EW - 1)
                    for sub in range(4):
                        nc.tensor.matmul(uts[sub],
                                         e_t[:, rg, sub * 128:(sub + 1) * 128],
                                         vt_sb[:, mc, :],
                                         start=first, stop=last)

            def issue_epilogue(g):
                uts = u_tiles.pop(g)
                # normalize per n-sub-block to bf16; transpose back to
                # [c, n] on the DMA xbar engine (keeps PE free); then
                # y = un_t + xp on DVE.
                if pe_transpose:
                    tph = [u_ps.tile([128, NG], dt.bfloat16, tag=f"u{cb}",
                                     name="tph") for cb in range(2)]
                else:
                    un_gt = upool.tile([128, 2, NG], dt.bfloat16, tag="ungt",
                                       name="un_gt")
                for sub in range(4):
                    ut = uts[sub]
                    rinv = scal.tile([128, 1], f32, tag="rinv", name="rinv")
                    nc.vector.reciprocal(rinv, ut[:, C:CP])
                    un = upool.tile([128, C], dt.bfloat16, tag="un", name="un")
                    nc.vector.tensor_scalar_mul(un, ut[:, 0:C], rinv)
                    if pe_transpose:
                        for cb in range(2):
                            nc.tensor.transpose(
                                tph[cb][:, sub * 128:(sub + 1) * 128],
                                un[:, cb * 128:(cb + 1) * 128], ident)
                    else:
                        nc.sync.dma_start_transpose(
                            out=un_gt[:, :, sub * 128:(sub + 1) * 128], in_=un)
                sl = slice(g * NG, (g + 1) * NG)
                for cb in range(2):
                    y_t = ypool.tile([128, NG], f32, tag="y", name="y")
                    src = tph[cb] if pe_transpose else un_gt[:, cb, :]
                    nc.vector.scalar_tensor_tensor(
                        y_t, src, bv_sb[:, cb:cb + 1], xhi_sb[:, cb, sl],
                        mybir.AluOpType.add, mybir.AluOpType.add)
                    nc.sync.dma_start(
                        out=y_d[:].rearrange("(cc p) m -> p cc m", p=128)[:, cb, sl],
                        in_=y_t,
                    )

            for g in range(G + 1):
                if g < G:
                    u_tiles[g] = [u_ps.tile([128, CP], f32, tag=f"u{s}", name=f"u{s}")
                                  for s in range(4)]
                for j in range(RND):
                    if g < G:
                        issue_lt_exp(g, j)
                    if g >= 1:
                        issue_av(g - 1, j)
                if g >= 1:
                    issue_epilogue(g - 1)

    nc.compile()
    return nc


def prep_in_maps(inputs):
    """Full inputs dict -> per-core in_maps for run_bass_kernel_spmd."""
    import ml_dtypes
    bf = ml_dtypes.bfloat16
    x = np.asarray(inputs["x"], dtype=np.float32)
    xr = np.ascontiguousarray(x.reshape(B, C, N))
    x_hi = xr.astype(bf)
    shared = {
        "wqT": np.ascontiguousarray(
            np.asarray(inputs["wq"], np.float32).T.astype(bf)),
        "wkT": np.ascontiguousarray(
            (np.float32(A_SCH)
             * np.asarray(inputs["wk"], np.float32)).T.astype(bf)),
        "wvT": np.ascontiguousarray(
            (np.asarray(inputs["gamma"], np.float32).reshape(())
             * np.asarray(inputs["wv"], np.float32)).T.astype(bf)),
        "bq": np.ascontiguousarray(
            np.asarray(inputs["bq"], np.float32).reshape(C8, 1)),
        "bk": np.ascontiguousarray(
            (np.float32(A_SCH)
             * np.asarray(inputs["bk"], np.float32)).reshape(C8, 1)),
        "gbv": np.ascontiguousarray(
            (np.asarray(inputs["gamma"], np.float32).reshape(())
             * np.asarray(inputs["bv"], np.float32)).reshape(2, 128).T),
    }
    return [dict(shared, x_hi=np.ascontiguousarray(x_hi[i]))
            for i in range(B)]


_nc_cache = None


def kernel(**inputs) -> np.ndarray:
    global _nc_cache
    if _nc_cache is None:
        _nc_cache = build_program()
    nc = _nc_cache
    in_maps = prep_in_maps(inputs)
    res = run_bass_kernel_spmd(nc, in_maps, core_ids=list(range(B)))
    y = np.stack([res.results[i]["y"] for i in range(B)], axis=0)
    return y.reshape(B, C, 64, 64).astype(np.float32)


if __name__ == "__main__":
    rng = np.random.default_rng(0)
    ins = {
        "x": rng.standard_normal((B, C, 64, 64), dtype=np.float32),
        "wq": rng.standard_normal((C8, C), dtype=np.float32) / 16,
        "bq": rng.standard_normal((C8,), dtype=np.float32) * 0.01,
        "wk": rng.standard_normal((C8, C), dtype=np.float32) / 16,
        "bk": rng.standard_normal((C8,), dtype=np.float32) * 0.01,
        "wv": rng.standard_normal((C, C), dtype=np.float32) / 16,
        "bv": rng.standard_normal((C,), dtype=np.float32) * 0.01,
        "gamma": rng.standard_normal((1,), dtype=np.float32) * 0.1,
    }
    out = kernel(**ins)
    print("kernel output", out.shape, out.dtype)


# revision 44
# speedup vs baseline: 1.4856x; 1.1554x over previous
"""Trainium2 Bass kernel for nn_Attention_7078106104284.

Self-attention block (SAGAN-style) over x[8, 256, 64, 64]:
  q = wq@x+bq [32,n], k = wk@x+bk [32,n], v = wv@x+bv [256,n], n = 4096
  attn = softmax(q^T k, axis=m);  y = x + gamma * (v @ attn^T)

Sharding: data-parallel over batch - one batch element per NeuronCore (8 cores).

All compute in plain bf16 (correctness gate is rel_err < 2e-2; bf16 inputs
with fp32 PSUM accumulation lands ~3e-3). Design notes:

  - q/k projections: K=256 bf16 accumulating matmuls per 512-wide n-tile,
    q and k col-tiled into disjoint 32-col PE strips (concurrent); bias
    fused into the DVE eviction; results replicated x4 along SBUF
    partitions (strips at 0/32/64/96) via SBUF->SBUF DMA so logit matmuls
    can be row-tiled. gamma is folded into wv/bv on the host; k is
    pre-scaled by a=128*log2e (Schraudolph-ready logits; ACT exp uses
    scale=1/a).
  - Logits Lt[m,n] = sum_o k[o,m] q[o,n]: K=32 matmuls packed 4-per-quad
    with tile_position=(32i,0) - four m-chunks compute concurrently in
    disjoint 32-row strips of the PE array, each into its own PSUM bank.
  - exp fused with PSUM->SBUF evacuation on ACT as ONE FD=2048 call per
    quad, bf16 out. Softmax max-subtraction skipped: |logit| < 50 << 88.
    (Optional dve_quads: exp via int16-Schraudolph on DVE - measured
    neutral under sustained load, default off.)
  - vT[m, c'] bf16 with a ones column at c' = 256 (DVE memset) so the
    softmax denominator Z rides the AV product for free.
  - AV transposed: uT[n, c'] = sum_m e[m, n] vT[m, c'] per 128-wide n-sub
    (stationary = e chunk w/ fast-weight-load, moving = vT, N=257).
  - Normalize on DVE: uTn = uT * (1/Z[n]) per-partition, bf16; PE
    transposes uTn back to [c, n] (transpose-mode); epilogue fuses
    y = tph + gamma*bv + x_hi in one scalar_tensor_tensor op (v's bias
    folds into the residual because sum_m attn = 1).

Measured pitfalls (see memory): GPSIMD is ~25x slower than DVE on large
tiles; DMA-xbar transposes serialize the epilogue (lose ~70us vs PE);
timing must be compared via same-session interleaved A/B (thermal drift).
"""

import sys

sys.path.insert(0, "/opt/trn_rl_repo")

import numpy as np
from contextlib import ExitStack

import concourse.bass as bass
import concourse.bacc as bacc
import concourse.tile as tile
import concourse.mybir as mybir
from concourse.masks import make_identity
from concourse.bass_utils import run_bass_kernel_spmd

dt = mybir.dt
AF = mybir.ActivationFunctionType

B = 8
C = 256
C8 = 32
N = 4096          # h*w spatial positions
NG = 512          # n-group width (one PSUM bank of fp32)
G = N // NG       # 8 n-groups
MC = N // 128     # 32 m-chunks
EW = 4            # m-chunks per quad (PSUM banks per plt tile)
RND = MC // EW    # quads (rounds) per group
CP = C + 1        # AV output channels incl. the Z ones-column
A_SCH = 128 * np.log2(np.e)   # Schraudolph logit pre-scale (folded into k)
# quads whose exp runs on DVE via the int16 Schraudolph trick. Measured:
# no benefit under sustained load (PE-bound; ACT has slack) - default off.
DVE_QUADS = frozenset()


def build_program(reps=1, ablate=(), dve_quads=DVE_QUADS, pe_transpose=True,
                  xp_engine="vector", ebufs=16, tph_pack=True):
    nc = bacc.Bacc("TRN2", target_bir_lowering=False)
    f32 = dt.float32
    bf16 = dt.bfloat16
    xhi_d = nc.declare_dram_parameter("x_hi", [C, N], bf16, isOutput=False)
    wq_d = nc.declare_dram_parameter("wqT", [C, C8], bf16, isOutput=False)
    wk_d = nc.declare_dram_parameter("wkT", [C, C8], bf16, isOutput=False)
    wv_d = nc.declare_dram_parameter("wvT", [C, C], bf16, isOutput=False)
    bq_d = nc.declare_dram_parameter("bq", [C8, 1], f32, isOutput=False)
    bk_d = nc.declare_dram_parameter("bk", [C8, 1], f32, isOutput=False)
    bv_d = nc.declare_dram_parameter("gbv", [128, 2], f32, isOutput=False)
    y_d = nc.declare_dram_parameter("y", [C, N], f32, isOutput=True)

    with tile.TileContext(nc) as tc, ExitStack() as ctx:
        sing = ctx.enter_context(tc.tile_pool(name="sing", bufs=1))
        epool = ctx.enter_context(tc.tile_pool(name="epool", bufs=ebufs))
        upool = ctx.enter_context(tc.tile_pool(name="upool", bufs=6))
        ypool = ctx.enter_context(tc.tile_pool(name="ypool", bufs=3))
        scal = ctx.enter_context(tc.tile_pool(name="scal", bufs=4))

        lt_ps = ctx.enter_context(tc.tile_pool(name="lt_ps", bufs=1, space="PSUM"))
        u_ps = ctx.enter_context(tc.tile_pool(name="u_ps", bufs=1, space="PSUM"))

        for _rep in range(reps):
            # ---- static inputs ----
            xhi_sb = sing.tile([128, 2, N], bf16)
            nc.sync.dma_start(out=xhi_sb, in_=xhi_d[:].rearrange("(cc p) m -> p cc m", p=128))
            wq_sb = sing.tile([128, 2, C8], bf16)        # slots cc: 0, 1
            nc.sync.dma_start(out=wq_sb, in_=wq_d[:].rearrange("(cc p) o -> p cc o", p=128))
            wk_sb = sing.tile([128, 2, C8], bf16)
            nc.sync.dma_start(out=wk_sb, in_=wk_d[:].rearrange("(cc p) o -> p cc o", p=128))
            wv_sb = sing.tile([128, 2, C], bf16)
            nc.sync.dma_start(out=wv_sb, in_=wv_d[:].rearrange("(cc p) c -> p cc c", p=128))
            bq_sb = sing.tile([C8, 1], f32)
            nc.sync.dma_start(out=bq_sb, in_=bq_d[:])
            bk_sb = sing.tile([C8, 1], f32)
            nc.sync.dma_start(out=bk_sb, in_=bk_d[:])
            bv_sb = sing.tile([128, 2], f32)   # gamma*bv, pre-scaled on host
            nc.sync.dma_start(out=bv_sb, in_=bv_d[:])

            # Schraudolph bias: e^l ~ bf16_bits(int16(l*128*log2e + 16256-C)).
            # k (and bk) are pre-scaled by a=128*log2e on the host, so PSUM
            # logits arrive as l*a; ACT-exp quads undo it via scale=1/a.
            bsch = sing.tile([128, 1], f32)
            nc.vector.memset(bsch, 16256.0 - 5.59)
            if pe_transpose:
                ident = sing.tile([128, 128], bf16)
                make_identity(nc, ident)

            # Residual y = attn_out + x_hi + gamma*bv is fused into the
            # epilogue via scalar_tensor_tensor (v's bias folds into the
            # residual because sum_m attn = 1) - no staged xp needed.

            # ---- q/k projections, replicated x4 along partition strips ----
            q4 = sing.tile([128, N], bf16)   # strips at partitions 0/32/64/96
            k4 = sing.tile([128, N], bf16)
            for s in range(G):
                sl = slice(s * NG, (s + 1) * NG)
                # q and k col-tiled into disjoint 32-col strips of the PE
                # array - the two M=32 matmuls run concurrently.
                pqk = u_ps.tile([2 * C8, NG], f32, tag="u0", name="pqk")
                for cc in range(2):
                    nc.tensor.matmul(pqk[0:C8, :], wq_sb[:, cc, :],
                                     xhi_sb[:, cc, sl],
                                     start=(cc == 0), stop=(cc == 1),
                                     tile_position=(0, 0))
                    nc.tensor.matmul(pqk[C8:2 * C8, :], wk_sb[:, cc, :],
                                     xhi_sb[:, cc, sl],
                                     start=(cc == 0), stop=(cc == 1),
                                     tile_position=(0, 32))
                nc.vector.tensor_scalar_add(q4[0:C8, sl], pqk[0:C8, :], bq_sb)
                nc.vector.tensor_scalar_add(k4[0:C8, sl], pqk[C8:2 * C8, :],
                                            bk_sb)
            # replicate to strips 1..3 (SBUF->SBUF DMA shifts partitions)
            for st in range(1, 4):
                nc.sync.dma_start(out=q4[st * C8:(st + 1) * C8, :], in_=q4[0:C8, :])
                nc.sync.dma_start(out=k4[st * C8:(st + 1) * C8, :], in_=k4[0:C8, :])

            # ---- vT[m, c'] in bf16 with ones column at c' = 256 ----
            vt_sb = sing.tile([128, MC, CP], bf16)
            nc.vector.memset(vt_sb[:, :, C:CP], 1.0)   # Z ones-column
            for mc in range(MC):
                msl = slice(mc * 128, (mc + 1) * 128)
                pv = u_ps.tile([128, C], f32, tag="u2", name="pv")
                for cc in range(2):
                    nc.tensor.matmul(pv, xhi_sb[:, cc, msl], wv_sb[:, cc, :],
                                     start=(cc == 0), stop=(cc == 1))
                nc.vector.tensor_copy(vt_sb[:, mc, 0:C], pv)

            # ---- attention, software-pipelined over n-groups ----
            e_tiles = {}
            u_tiles = {}

            def issue_lt_exp(g, j):
                # quad j: logits for m-chunks 4j..4j+3, row-tiled 4x
                # concurrent (tile_position=(32i,0)), each into its own
                # PSUM bank; one fused FD=2048 exp over all four.
                sl = slice(g * NG, (g + 1) * NG)
                plt = lt_ps.tile([128, EW, NG], f32, tag="plt", name="plt")
                for rg in range(EW if "lt" not in ablate else 1):
                    mc = EW * j + rg
                    msl = slice(mc * 128, (mc + 1) * 128)
                    nc.tensor.matmul(plt[:, rg, :],
                                     k4[rg * C8:(rg + 1) * C8, msl],
                                     q4[rg * C8:(rg + 1) * C8, sl],
                                     start=True, stop=True,
                                     tile_position=(rg * C8, 0))
                e_t = epool.tile([128, EW, NG], bf16, tag="e", name="e_t")
                if "noexp" in ablate:
                    nc.gpsimd.memset(e_t, 0.5)
                elif (g, j) in dve_quads:
                    nc.vector.tensor_scalar_add(
                        e_t[:, :, :].bitcast(dt.int16), plt, bsch)
                else:
                    fn = AF.Exp if "exp" not in ablate else AF.Copy
                    nc.scalar.activation(e_t, plt, fn, scale=float(1.0 / A_SCH))
                e_tiles[(g, j)] = e_t

            def issue_av(g, j):
                uts = u_tiles[g]
                e_t = e_tiles.pop((g, j))
                if "av" in ablate:
                    if j == 0:
                        for sub in range(4):
                            nc.tensor.matmul(uts[sub],
                                             e_t[:, 0, sub * 128:(sub + 1) * 128],
                                             vt_sb[:, 0, :], start=True, stop=True)
                    return
                for rg in range(EW):
                    mc = EW * j + rg
                    first = (j == 0 and rg == 0)
                    last = (j == RND - 1 and rg == EW - 1)
                    for sub in range(4):
                        nc.tensor.matmul(uts[sub],
                                         e_t[:, rg, sub * 128:(sub + 1) * 128],
                                         vt_sb[:, mc, :],
                                         start=first, stop=last)

            def issue_epilogue(g):
                uts = u_tiles.pop(g)
                # normalize per n-sub-block to bf16; transpose back to
                # [c, n] on the DMA xbar engine (keeps PE free); then
                # y = un_t + xp on DVE.
                if pe_transpose and tph_pack:
                    # both transpose halves packed into ONE psum bank (tag
                    # u0, bf16 2KB) so banks u1-u3 free for the next
                    # group's AV right after the early normalize reads.
                    tphp = u_ps.tile([128, 2, NG], dt.bfloat16, tag="u0",
                                     name="tphp")
                    tph = [tphp[:, cb, :] for cb in range(2)]
                elif pe_transpose:
                    tph = [u_ps.tile([128, NG], dt.bfloat16, tag=f"u{cb}",
                                     name="tph") for cb in range(2)]
                else:
                    un_gt = upool.tile([128, 2, NG], dt.bfloat16, tag="ungt",
                                       name="un_gt")
                for sub in range(4):
                    ut = uts[sub]
                    rinv = scal.tile([128, 1], f32, tag="rinv", name="rinv")
                    nc.vector.reciprocal(rinv, ut[:, C:CP])
                    un = upool.tile([128, C], dt.bfloat16, tag="un", name="un")
                    nc.vector.tensor_scalar_mul(un, ut[:, 0:C], rinv)
                    if pe_transpose:
                        for cb in range(2):
                            nc.tensor.transpose(
                                tph[cb][:, sub * 128:(sub + 1) * 128],
                                un[:, cb * 128:(cb + 1) * 128], ident)
                    else:
                        nc.sync.dma_start_transpose(
                            out=un_gt[:, :, sub * 128:(sub + 1) * 128], in_=un)
                sl = slice(g * NG, (g + 1) * NG)
                for cb in range(2):
                    y_t = ypool.tile([128, NG], f32, tag="y", name="y")
                    src = tph[cb] if pe_transpose else un_gt[:, cb, :]
                    nc.vector.scalar_tensor_tensor(
                        y_t, src, bv_sb[:, cb:cb + 1], xhi_sb[:, cb, sl],
                        mybir.AluOpType.add, mybir.AluOpType.add)
                    nc.sync.dma_start(
                        out=y_d[:].rearrange("(cc p) m -> p cc m", p=128)[:, cb, sl],
                        in_=y_t,
                    )

            for g in range(G + 1):
                if g < G:
                    u_tiles[g] = [u_ps.tile([128, CP], f32, tag=f"u{s}", name=f"u{s}")
                                  for s in range(4)]
                for j in range(RND):
                    if g < G:
                        issue_lt_exp(g, j)
                    if g >= 1:
                        issue_av(g - 1, j)
                if g >= 1:
                    issue_epilogue(g - 1)

    nc.compile()
    return nc


def prep_in_maps(inputs):
    """Full inputs dict -> per-core in_maps for run_bass_kernel_spmd."""
    import ml_dtypes
    bf = ml_dtypes.bfloat16
    x = np.asarray(inputs["x"], dtype=np.float32)
    xr = np.ascontiguousarray(x.reshape(B, C, N))
    x_hi = xr.astype(bf)
    shared = {
        "wqT": np.ascontiguousarray(
            np.asarray(inputs["wq"], np.float32).T.astype(bf)),
        "wkT": np.ascontiguousarray(
            (np.float32(A_SCH)
             * np.asarray(inputs["wk"], np.float32)).T.astype(bf)),
        "wvT": np.ascontiguousarray(
            (np.asarray(inputs["gamma"], np.float32).reshape(())
             * np.asarray(inputs["wv"], np.float32)).T.astype(bf)),
        "bq": np.ascontiguousarray(
            np.asarray(inputs["bq"], np.float32).reshape(C8, 1)),
        "bk": np.ascontiguousarray(
            (np.float32(A_SCH)
             * np.asarray(inputs["bk"], np.float32)).reshape(C8, 1)),
        "gbv": np.ascontiguousarray(
            (np.asarray(inputs["gamma"], np.float32).reshape(())
             * np.asarray(inputs["bv"], np.float32)).reshape(2, 128).T),
    }
    return [dict(shared, x_hi=np.ascontiguousarray(x_hi[i]))
            for i in range(B)]


_nc_cache = None


def kernel(**inputs) -> np.ndarray:
    global _nc_cache
    if _nc_cache is None:
        _nc_cache = build_program()
    nc = _nc_cache
    in_maps = prep_in_maps(inputs)
    res = run_bass_kernel_spmd(nc, in_maps, core_ids=list(range(B)))
    y = np.stack([res.results[i]["y"] for i in range(B)], axis=0)
    return y.reshape(B, C, 64, 64).astype(np.float32)


if __name__ == "__main__":
    rng = np.random.default_rng(0)
    ins = {
        "x": rng.standard_normal((B, C, 64, 64), dtype=np.float32),
        "wq": rng.standard_normal((C8, C), dtype=np.float32) / 16,
        "bq": rng.standard_normal((C8,), dtype=np.float32) * 0.01,
        "wk": rng.standard_normal((C8, C), dtype=np.float32) / 16,
        "bk": rng.standard_normal((C8,), dtype=np.float32) * 0.01,
        "wv": rng.standard_normal((C, C), dtype=np.float32) / 16,
        "bv": rng.standard_normal((C,), dtype=np.float32) * 0.01,
        "gamma": rng.standard_normal((1,), dtype=np.float32) * 0.1,
    }
    out = kernel(**ins)
    print("kernel output", out.shape, out.dtype)


# revision 46
# speedup vs baseline: 1.5553x; 1.0469x over previous
"""Trainium2 Bass kernel for nn_Attention_7078106104284.

Self-attention block (SAGAN-style) over x[8, 256, 64, 64]:
  q = wq@x+bq [32,n], k = wk@x+bk [32,n], v = wv@x+bv [256,n], n = 4096
  attn = softmax(q^T k, axis=m);  y = x + gamma * (v @ attn^T)

Sharding: data-parallel over batch - one batch element per NeuronCore (8 cores).

All compute in plain bf16 (correctness gate is rel_err < 2e-2; bf16 inputs
with fp32 PSUM accumulation lands ~3e-3). Design notes:

  - q/k projections: K=256 bf16 accumulating matmuls per 512-wide n-tile,
    q and k col-tiled into disjoint 32-col PE strips (concurrent); bias
    fused into the DVE eviction; results replicated x4 along SBUF
    partitions (strips at 0/32/64/96) via SBUF->SBUF DMA so logit matmuls
    can be row-tiled. gamma is folded into wv/bv on the host; k is
    pre-scaled by a=128*log2e (Schraudolph-ready logits; ACT exp uses
    scale=1/a).
  - Logits Lt[m,n] = sum_o k[o,m] q[o,n]: K=32 matmuls packed 4-per-quad
    with tile_position=(32i,0) - four m-chunks compute concurrently in
    disjoint 32-row strips of the PE array, each into its own PSUM bank.
  - exp fused with PSUM->SBUF evacuation on ACT as ONE FD=2048 call per
    quad, bf16 out. Softmax max-subtraction skipped: |logit| < 50 << 88.
    (Optional dve_quads: exp via int16-Schraudolph on DVE - measured
    neutral under sustained load, default off.)
  - vT[m, c'] bf16 with a ones column at c' = 256 (DVE memset) so the
    softmax denominator Z rides the AV product for free.
  - AV transposed: uT[n, c'] = sum_m e[m, n] vT[m, c'] per 128-wide n-sub
    (stationary = e chunk w/ fast-weight-load, moving = vT, N=257).
  - Normalize on DVE: uTn = uT * (1/Z[n]) per-partition, bf16; PE
    transposes uTn back to [c, n] (transpose-mode); epilogue fuses
    y = tph + gamma*bv + x_hi in one scalar_tensor_tensor op (v's bias
    folds into the residual because sum_m attn = 1).

Measured pitfalls (see memory): GPSIMD is ~25x slower than DVE on large
tiles; DMA-xbar transposes serialize the epilogue (lose ~70us vs PE);
timing must be compared via same-session interleaved A/B (thermal drift).
"""

import sys

sys.path.insert(0, "/opt/trn_rl_repo")

import numpy as np
from contextlib import ExitStack

import concourse.bass as bass
import concourse.bacc as bacc
import concourse.tile as tile
import concourse.mybir as mybir
from concourse.masks import make_identity
from concourse.bass_utils import run_bass_kernel_spmd

dt = mybir.dt
AF = mybir.ActivationFunctionType

B = 8
C = 256
C8 = 32
N = 4096          # h*w spatial positions
NG = 512          # n-group width (one PSUM bank of fp32)
G = N // NG       # 8 n-groups
MC = N // 128     # 32 m-chunks
EW = 4            # m-chunks per quad (PSUM banks per plt tile)
RND = MC // EW    # quads (rounds) per group
CP = C + 1        # AV output channels incl. the Z ones-column
A_SCH = 128 * np.log2(np.e)   # Schraudolph logit pre-scale (folded into k)
# quads whose exp runs on DVE via the int16 Schraudolph trick. Measured:
# no benefit under sustained load (PE-bound; ACT has slack) - default off.
DVE_QUADS = frozenset()


def build_program(reps=1, ablate=(), dve_quads=DVE_QUADS, pe_transpose=True,
                  xp_engine="vector", ebufs=16, tph_pack=True):
    nc = bacc.Bacc("TRN2", target_bir_lowering=False)
    f32 = dt.float32
    bf16 = dt.bfloat16
    xhi_d = nc.declare_dram_parameter("x_hi", [C, N], bf16, isOutput=False)
    wq_d = nc.declare_dram_parameter("wqT", [C, C8], bf16, isOutput=False)
    wk_d = nc.declare_dram_parameter("wkT", [C, C8], bf16, isOutput=False)
    wv_d = nc.declare_dram_parameter("wvT", [C, C], bf16, isOutput=False)
    bq_d = nc.declare_dram_parameter("bq", [C8, 1], f32, isOutput=False)
    bk_d = nc.declare_dram_parameter("bk", [C8, 1], f32, isOutput=False)
    bv_d = nc.declare_dram_parameter("gbv", [128, 2], f32, isOutput=False)
    y_d = nc.declare_dram_parameter("y", [C, N], f32, isOutput=True)

    with tile.TileContext(nc) as tc, ExitStack() as ctx:
        sing = ctx.enter_context(tc.tile_pool(name="sing", bufs=1))
        # x_hi is read by the fused-residual epilogues until the very end
        # of each rep; double-buffer it so the next rep's input DMA and
        # projections overlap the current rep's tail.
        xpool = ctx.enter_context(tc.tile_pool(name="xpool", bufs=2))
        epool = ctx.enter_context(tc.tile_pool(name="epool", bufs=ebufs))
        upool = ctx.enter_context(tc.tile_pool(name="upool", bufs=6))
        ypool = ctx.enter_context(tc.tile_pool(name="ypool", bufs=3))
        scal = ctx.enter_context(tc.tile_pool(name="scal", bufs=4))

        lt_ps = ctx.enter_context(tc.tile_pool(name="lt_ps", bufs=1, space="PSUM"))
        u_ps = ctx.enter_context(tc.tile_pool(name="u_ps", bufs=1, space="PSUM"))

        for _rep in range(reps):
            # ---- static inputs ----
            xhi_sb = xpool.tile([128, 2, N], bf16, tag="xhi", name="xhi_sb")
            nc.sync.dma_start(out=xhi_sb, in_=xhi_d[:].rearrange("(cc p) m -> p cc m", p=128))
            wq_sb = sing.tile([128, 2, C8], bf16)        # slots cc: 0, 1
            nc.sync.dma_start(out=wq_sb, in_=wq_d[:].rearrange("(cc p) o -> p cc o", p=128))
            wk_sb = sing.tile([128, 2, C8], bf16)
            nc.sync.dma_start(out=wk_sb, in_=wk_d[:].rearrange("(cc p) o -> p cc o", p=128))
            wv_sb = sing.tile([128, 2, C], bf16)
            nc.sync.dma_start(out=wv_sb, in_=wv_d[:].rearrange("(cc p) c -> p cc c", p=128))
            bq_sb = sing.tile([C8, 1], f32)
            nc.sync.dma_start(out=bq_sb, in_=bq_d[:])
            bk_sb = sing.tile([C8, 1], f32)
            nc.sync.dma_start(out=bk_sb, in_=bk_d[:])
            bv_sb = sing.tile([128, 2], f32)   # gamma*bv, pre-scaled on host
            nc.sync.dma_start(out=bv_sb, in_=bv_d[:])

            # Schraudolph bias: e^l ~ bf16_bits(int16(l*128*log2e + 16256-C)).
            # k (and bk) are pre-scaled by a=128*log2e on the host, so PSUM
            # logits arrive as l*a; ACT-exp quads undo it via scale=1/a.
            bsch = sing.tile([128, 1], f32)
            nc.vector.memset(bsch, 16256.0 - 5.59)
            if pe_transpose:
                ident = sing.tile([128, 128], bf16)
                make_identity(nc, ident)

            # Residual y = attn_out + x_hi + gamma*bv is fused into the
            # epilogue via scalar_tensor_tensor (v's bias folds into the
            # residual because sum_m attn = 1) - no staged xp needed.

            # ---- q/k projections, replicated x4 along partition strips ----
            q4 = sing.tile([128, N], bf16)   # strips at partitions 0/32/64/96
            k4 = sing.tile([128, N], bf16)
            for s in range(G):
                sl = slice(s * NG, (s + 1) * NG)
                # q and k col-tiled into disjoint 32-col strips of the PE
                # array - the two M=32 matmuls run concurrently.
                pqk = u_ps.tile([2 * C8, NG], f32, tag="u0", name="pqk")
                for cc in range(2):
                    nc.tensor.matmul(pqk[0:C8, :], wq_sb[:, cc, :],
                                     xhi_sb[:, cc, sl],
                                     start=(cc == 0), stop=(cc == 1),
                                     tile_position=(0, 0))
                    nc.tensor.matmul(pqk[C8:2 * C8, :], wk_sb[:, cc, :],
                                     xhi_sb[:, cc, sl],
                                     start=(cc == 0), stop=(cc == 1),
                                     tile_position=(0, 32))
                nc.vector.tensor_scalar_add(q4[0:C8, sl], pqk[0:C8, :], bq_sb)
                nc.vector.tensor_scalar_add(k4[0:C8, sl], pqk[C8:2 * C8, :],
                                            bk_sb)
            # replicate to strips 1..3 (SBUF->SBUF DMA shifts partitions)
            for st in range(1, 4):
                nc.sync.dma_start(out=q4[st * C8:(st + 1) * C8, :], in_=q4[0:C8, :])
                nc.sync.dma_start(out=k4[st * C8:(st + 1) * C8, :], in_=k4[0:C8, :])

            # ---- vT[m, c'] in bf16 with ones column at c' = 256 ----
            vt_sb = sing.tile([128, MC, CP], bf16)
            nc.vector.memset(vt_sb[:, :, C:CP], 1.0)   # Z ones-column
            for mc in range(MC):
                msl = slice(mc * 128, (mc + 1) * 128)
                pv = u_ps.tile([128, C], f32, tag="u2", name="pv")
                for cc in range(2):
                    nc.tensor.matmul(pv, xhi_sb[:, cc, msl], wv_sb[:, cc, :],
                                     start=(cc == 0), stop=(cc == 1))
                nc.vector.tensor_copy(vt_sb[:, mc, 0:C], pv)

            # ---- attention, software-pipelined over n-groups ----
            e_tiles = {}
            u_tiles = {}

            def issue_lt_exp(g, j):
                # quad j: logits for m-chunks 4j..4j+3, row-tiled 4x
                # concurrent (tile_position=(32i,0)), each into its own
                # PSUM bank; one fused FD=2048 exp over all four.
                sl = slice(g * NG, (g + 1) * NG)
                plt = lt_ps.tile([128, EW, NG], f32, tag="plt", name="plt")
                for rg in range(EW if "lt" not in ablate else 1):
                    mc = EW * j + rg
                    msl = slice(mc * 128, (mc + 1) * 128)
                    nc.tensor.matmul(plt[:, rg, :],
                                     k4[rg * C8:(rg + 1) * C8, msl],
                                     q4[rg * C8:(rg + 1) * C8, sl],
                                     start=True, stop=True,
                                     tile_position=(rg * C8, 0))
                e_t = epool.tile([128, EW, NG], bf16, tag="e", name="e_t")
                if "noexp" in ablate:
                    nc.gpsimd.memset(e_t, 0.5)
                elif (g, j) in dve_quads:
                    nc.vector.tensor_scalar_add(
                        e_t[:, :, :].bitcast(dt.int16), plt, bsch)
                else:
                    fn = AF.Exp if "exp" not in ablate else AF.Copy
                    nc.scalar.activation(e_t, plt, fn, scale=float(1.0 / A_SCH))
                e_tiles[(g, j)] = e_t

            def issue_av(g, j):
                uts = u_tiles[g]
                e_t = e_tiles.pop((g, j))
                if "av" in ablate:
                    if j == 0:
                        for sub in range(4):
                            nc.tensor.matmul(uts[sub],
                                             e_t[:, 0, sub * 128:(sub + 1) * 128],
                                             vt_sb[:, 0, :], start=True, stop=True)
                    return
                for rg in range(EW):
                    mc = EW * j + rg
                    first = (j == 0 and rg == 0)
                    last = (j == RND - 1 and rg == EW - 1)
                    for sub in range(4):
                        nc.tensor.matmul(uts[sub],
                                         e_t[:, rg, sub * 128:(sub + 1) * 128],
                                         vt_sb[:, mc, :],
                                         start=first, stop=last)

            def issue_epilogue(g):
                uts = u_tiles.pop(g)
                # normalize per n-sub-block to bf16; transpose back to
                # [c, n] on the DMA xbar engine (keeps PE free); then
                # y = un_t + xp on DVE.
                if pe_transpose and tph_pack:
                    # both transpose halves packed into ONE psum bank (tag
                    # u0, bf16 2KB) so banks u1-u3 free for the next
                    # group's AV right after the early normalize reads.
                    tphp = u_ps.tile([128, 2, NG], dt.bfloat16, tag="u0",
                                     name="tphp")
                    tph = [tphp[:, cb, :] for cb in range(2)]
                elif pe_transpose:
                    tph = [u_ps.tile([128, NG], dt.bfloat16, tag=f"u{cb}",
                                     name="tph") for cb in range(2)]
                else:
                    un_gt = upool.tile([128, 2, NG], dt.bfloat16, tag="ungt",
                                       name="un_gt")
                for sub in range(4):
                    ut = uts[sub]
                    rinv = scal.tile([128, 1], f32, tag="rinv", name="rinv")
                    nc.vector.reciprocal(rinv, ut[:, C:CP])
                    un = upool.tile([128, C], dt.bfloat16, tag="un", name="un")
                    nc.vector.tensor_scalar_mul(un, ut[:, 0:C], rinv)
                    if pe_transpose:
                        for cb in range(2):
                            nc.tensor.transpose(
                                tph[cb][:, sub * 128:(sub + 1) * 128],
                                un[:, cb * 128:(cb + 1) * 128], ident)
                    else:
                        nc.sync.dma_start_transpose(
                            out=un_gt[:, :, sub * 128:(sub + 1) * 128], in_=un)
                sl = slice(g * NG, (g + 1) * NG)
                for cb in range(2):
                    y_t = ypool.tile([128, NG], f32, tag="y", name="y")
                    src = tph[cb] if pe_transpose else un_gt[:, cb, :]
                    nc.vector.scalar_tensor_tensor(
                        y_t, src, bv_sb[:, cb:cb + 1], xhi_sb[:, cb, sl],
                        mybir.AluOpType.add, mybir.AluOpType.add)
                    nc.sync.dma_start(
                        out=y_d[:].rearrange("(cc p) m -> p cc m", p=128)[:, cb, sl],
                        in_=y_t,
                    )

            for g in range(G + 1):
                if g < G:
                    u_tiles[g] = [u_ps.tile([128, CP], f32, tag=f"u{s}", name=f"u{s}")
                                  for s in range(4)]
                for j in range(RND):
                    if g < G:
                        issue_lt_exp(g, j)
                    if g >= 1:
                        issue_av(g - 1, j)
                if g >= 1:
                    issue_epilogue(g - 1)

    nc.compile()
    return nc


def prep_in_maps(inputs):
    """Full inputs dict -> per-core in_maps for run_bass_kernel_spmd."""
    import ml_dtypes
    bf = ml_dtypes.bfloat16
    x = np.asarray(inputs["x"], dtype=np.float32)
    xr = np.ascontiguousarray(x.reshape(B, C, N))
    x_hi = xr.astype(bf)
    shared = {
        "wqT": np.ascontiguousarray(
            np.asarray(inputs["wq"], np.float32).T.astype(bf)),
        "wkT": np.ascontiguousarray(
            (np.float32(A_SCH)
             * np.asarray(inputs["wk"], np.float32)).T.astype(bf)),
        "wvT": np.ascontiguousarray(
            (np.asarray(inputs["gamma"], np.float32).reshape(())
             * np.asarray(inputs["wv"], np.float32)).T.astype(bf)),
        "bq": np.ascontiguousarray(
            np.asarray(inputs["bq"], np.float32).reshape(C8, 1)),
        "bk": np.ascontiguousarray(
            (np.float32(A_SCH)
             * np.asarray(inputs["bk"], np.float32)).reshape(C8, 1)),
        "gbv": np.ascontiguousarray(
            (np.asarray(inputs["gamma"], np.float32).reshape(())
             * np.asarray(inputs["bv"], np.float32)).reshape(2, 128).T),
    }
    return [dict(shared, x_hi=np.ascontiguousarray(x_hi[i]))
            for i in range(B)]


_nc_cache = None


def kernel(**inputs) -> np.ndarray:
    global _nc_cache
    if _nc_cache is None:
        _nc_cache = build_program()
    nc = _nc_cache
    in_maps = prep_in_maps(inputs)
    res = run_bass_kernel_spmd(nc, in_maps, core_ids=list(range(B)))
    y = np.stack([res.results[i]["y"] for i in range(B)], axis=0)
    return y.reshape(B, C, 64, 64).astype(np.float32)


if __name__ == "__main__":
    rng = np.random.default_rng(0)
    ins = {
        "x": rng.standard_normal((B, C, 64, 64), dtype=np.float32),
        "wq": rng.standard_normal((C8, C), dtype=np.float32) / 16,
        "bq": rng.standard_normal((C8,), dtype=np.float32) * 0.01,
        "wk": rng.standard_normal((C8, C), dtype=np.float32) / 16,
        "bk": rng.standard_normal((C8,), dtype=np.float32) * 0.01,
        "wv": rng.standard_normal((C, C), dtype=np.float32) / 16,
        "bv": rng.standard_normal((C,), dtype=np.float32) * 0.01,
        "gamma": rng.standard_normal((1,), dtype=np.float32) * 0.1,
    }
    out = kernel(**ins)
    print("kernel output", out.shape, out.dtype)
